# revision 3
# baseline (speedup 1.0000x reference)
"""MoE top-2 routing + expert FFN for Trainium2, expert-parallel across 8 cores.

v4: bf16, weights SBUF-resident (loaded once per launch), piece pipeline with
    xt prefetch ordered ahead of the y-out DMA in the SP HWDGE ring, y copies
    on VectorE (ScalarE keeps only gelu), and 2-bank fused gelu when b1 == 0.
"""

import numpy as np
from contextlib import ExitStack

import ml_dtypes

B, S, D = 4, 2048, 1024
E, H, TOP_K = 8, 4096, 2
T = B * S
P = 128
KS1 = D // P   # 8
M2 = H // P    # 32
DC = D // P    # 8
BF16 = ml_dtypes.bfloat16


def _routing(xf, Wr, br):
    import jax
    import jax.numpy as jnp

    cpu = jax.local_devices(backend="cpu")[0]
    with jax.default_device(cpu):
        gate = jax.nn.softmax(jnp.asarray(xf) @ jnp.asarray(Wr) + jnp.asarray(br), axis=-1)
        top_w, top_i = jax.lax.top_k(gate, TOP_K)
        top_w = top_w / jnp.sum(top_w, axis=-1, keepdims=True)
    return np.asarray(top_i), np.asarray(top_w)


def _pieces(max_count):
    n_p = -(-max_count // 512)
    w = -(-max_count // (n_p * 8)) * 8
    widths = [w] * (n_p - 1)
    last = max_count - w * (n_p - 1)
    widths.append(-(-last // 8) * 8)
    out, off = [], 0
    for pn in widths:
        out.append((off, pn))
        off += pn
    return out, off


def _build_program(pieces, C, repeats, b1_zero, hw_loop=True):
    import concourse.tile as tile
    from concourse import bacc, mybir

    F32 = mybir.dt.float32
    BF = mybir.dt.bfloat16
    AF = mybir.ActivationFunctionType

    nc = bacc.Bacc("TRN2", target_bir_lowering=False, debug=False, num_devices=E)

    xt_ap = nc.dram_tensor("xt", [P, KS1, C], BF, kind="ExternalInput").ap()
    w1_ap = nc.dram_tensor("w1", [P, M2, KS1, P], BF, kind="ExternalInput").ap()
    w2_ap = nc.dram_tensor("w2", [P, DC, M2, P], BF, kind="ExternalInput").ap()
    b1_ap = nc.dram_tensor("b1", [M2, P], F32, kind="ExternalInput").ap()
    b2_ap = nc.dram_tensor("b2", [DC, P], F32, kind="ExternalInput").ap()
    y_ap = nc.dram_tensor("y", [P, DC, C], BF, kind="ExternalOutput").ap()

    with tile.TileContext(nc) as tc, ExitStack() as ctx:
        w1_pool = ctx.enter_context(tc.tile_pool(name="w1", bufs=1))
        w2_pool = ctx.enter_context(tc.tile_pool(name="w2", bufs=1))
        bias_pool = ctx.enter_context(tc.tile_pool(name="bias", bufs=1))
        xt_pool = ctx.enter_context(tc.tile_pool(name="xt", bufs=2))
        h1_pool = ctx.enter_context(tc.tile_pool(name="h1", bufs=1))
        y_pool = ctx.enter_context(tc.tile_pool(name="y", bufs=2))
        psA = ctx.enter_context(tc.tile_pool(name="psA", bufs=2, space="PSUM"))
        psB = ctx.enter_context(tc.tile_pool(name="psB", bufs=4, space="PSUM"))

        # Constants: loaded once per launch, SBUF-resident across repeats.
        w1t = w1_pool.tile([P, M2, KS1, P], BF, tag="w1", name="w1t")
        nc.sync.dma_start(w1t[:], w1_ap)
        w2t = w2_pool.tile([P, DC, M2, P], BF, tag="w2", name="w2t")
        nc.sync.dma_start(w2t[:], w2_ap)
        b1t = bias_pool.tile([P, M2], F32, tag="b1", name="b1t")
        nc.sync.dma_start(b1t[:], b1_ap.rearrange("m p -> p m"))
        b2t = bias_pool.tile([P, DC], F32, tag="b2", name="b2t")
        nc.sync.dma_start(b2t[:], b2_ap.rearrange("d p -> p d"))

        def load_xt(t0, pn):
            xt = xt_pool.tile([P, KS1, pn], BF, tag="xt", name="xt")
            nc.sync.dma_start(xt[:], xt_ap[:, :, t0 : t0 + pn])
            return xt

        def body():
            xts = [None] * len(pieces)
            xts[0] = load_xt(*pieces[0])
            for pi, (t0, pn) in enumerate(pieces):
                xt = xts[pi]
                h1 = h1_pool.tile([P, M2, pn], BF, tag="h1", name="h1")
                if b1_zero:
                    for m2 in range(M2 // 2):
                        ps = psA.tile([P, 2, 512], F32, tag="psA", name="psA")
                        for half in range(2):
                            m = 2 * m2 + half
                            for k in range(KS1):
                                nc.tensor.matmul(
                                    ps[:, half, :pn],
                                    w1t[:, m, k, :],
                                    xt[:, k, :],
                                    start=(k == 0),
                                    stop=(k == KS1 - 1),
                                )
                        nc.scalar.activation(
                            h1[:, 2 * m2 : 2 * m2 + 2, :],
                            ps[:, :, :pn],
                            AF.Gelu,
                        )
                else:
                    for m in range(M2):
                        ps = psA.tile([P, 2, 512], F32, tag="psA", name="psA")
                        pss = ps[:, m % 2, :pn]
                        for k in range(KS1):
                            nc.tensor.matmul(
                                pss,
                                w1t[:, m, k, :],
                                xt[:, k, :],
                                start=(k == 0),
                                stop=(k == KS1 - 1),
                            )
                        nc.scalar.activation(
                            h1[:, m, :], pss, AF.Gelu, bias=b1t[:, m : m + 1]
                        )
                # prefetch next piece's tokens ahead of the y DMA in the ring
                if pi + 1 < len(pieces):
                    xts[pi + 1] = load_xt(*pieces[pi + 1])
                yt = y_pool.tile([P, DC, pn], BF, tag="y", name="yt")
                for d in range(DC):
                    ps = psB.tile([P, 512], F32, tag="psB", name="psB")[:, :pn]
                    for m in range(M2):
                        nc.tensor.matmul(
                            ps,
                            w2t[:, d, m, :],
                            h1[:, m, :],
                            start=(m == 0),
                            stop=(m == M2 - 1),
                        )
                    nc.vector.tensor_tensor(
                        yt[:, d, :],
                        ps,
                        b2t[:, d : d + 1].to_broadcast([P, pn]),
                        mybir.AluOpType.add,
                    )
                nc.sync.dma_start(y_ap[:, :, t0 : t0 + pn], yt[:])

        if repeats > 1 and hw_loop:
            with tc.For_i(0, repeats, 1):
                body()
        elif repeats > 1:
            for _ in range(repeats):
                body()
        else:
            body()

    nc.finalize()
    return nc


def _pack_inputs(xf, W1, b1, W2, b2, top_i, top_w, C):
    in_maps = []
    idx_list = []
    w_list = []
    for e in range(E):
        sel = (top_i == e).any(axis=1)
        idx = np.nonzero(sel)[0]
        we = (top_w * (top_i == e)).sum(axis=1)[idx].astype(np.float32)
        idx_list.append(idx)
        w_list.append(we)

        n = len(idx)
        Xg = np.zeros((C, D), dtype=np.float32)
        Xg[:n] = xf[idx]
        xt = Xg.reshape(C, KS1, P).transpose(2, 1, 0).astype(BF16)
        w1p = W1[e].reshape(KS1, P, M2, P).transpose(1, 2, 0, 3).astype(BF16)
        w2p = W2[e].reshape(M2, P, DC, P).transpose(1, 2, 0, 3).astype(BF16)
        b1p = np.ascontiguousarray(b1[e].reshape(M2, P))
        b2p = np.ascontiguousarray(b2[e].reshape(DC, P))

        in_maps.append({"xt": xt, "w1": w1p, "w2": w2p, "b1": b1p, "b2": b2p})
    return in_maps, idx_list, w_list


def _run(x, Wr, br, W1, b1, W2, b2, repeats=1, timing_runs=0, trace=False):
    import time

    from concourse.bass_utils import run_bass_kernel_spmd

    x = np.asarray(x, dtype=np.float32)
    Wr = np.asarray(Wr, dtype=np.float32)
    br = np.asarray(br, dtype=np.float32)
    W1 = np.asarray(W1, dtype=np.float32)
    b1 = np.asarray(b1, dtype=np.float32)
    W2 = np.asarray(W2, dtype=np.float32)
    b2 = np.asarray(b2, dtype=np.float32)

    xf = x.reshape(T, D)
    top_i, top_w = _routing(xf, Wr, br)

    counts = np.bincount(top_i.ravel(), minlength=E)
    pieces, C = _pieces(int(counts.max()))

    nc = _build_program(pieces, C, repeats, b1_zero=not np.any(b1))
    in_maps, idx_list, w_list = _pack_inputs(xf, W1, b1, W2, b2, top_i, top_w, C)

    res = run_bass_kernel_spmd(nc, in_maps, core_ids=list(range(E)), trace=trace)

    aux = []
    for _ in range(timing_runs):
        t0 = time.perf_counter()
        run_bass_kernel_spmd(nc, in_maps, core_ids=list(range(E)))
        aux.append(time.perf_counter() - t0)

    out = np.zeros((T, D), dtype=np.float32)
    for e in range(E):
        idx = idx_list[e]
        n = len(idx)
        if n == 0:
            continue
        yp = np.asarray(res.results[e]["y"]).astype(np.float32)
        Ye = yp.transpose(2, 1, 0).reshape(-1, D)  # [C, D]
        out[idx] += w_list[e][:, None] * Ye[:n]

    return out.reshape(B, S, D), aux


def kernel(x, Wr, br, W1, b1, W2, b2):
    out, _ = _run(x, Wr, br, W1, b1, W2, b2, repeats=1)
    return out

